# revision 21
# baseline (speedup 1.0000x reference)
# CoAttention Trainium2 kernel.
#
# Reference computation (B=4, LC=512, LW=256, H=512):
#   c = char @ Wc.T + Wc_b                     [B, LC, H]
#   w = word @ Ww.T + Ww_b                     [B, LW, H]
#   scores = einsum('bcwh,h->bcw', tanh(c[:,:,None,:] + w[:,None,:,:]), Wv) + Wv_b
#   char_att = softmax(scores, axis=2); word_att = softmax(scores, axis=1)
#   char_context = char_att @ word;  word_context = word_att.T @ char
#
# Sharding: 8 cores = 4 batches x 2 word-halves. Each core owns (b, 128 words,
# all 512 chars). The word-softmax (over chars) is fully local; the
# char-softmax (over words) is combined on the host from per-half partial
# exp-sums (softmax is shift-invariant and scores are O(5), so no max
# subtraction is needed; Wv_b shifts all scores equally and cancels).
#
# Per-core device pipeline:
#   PE:  cpT[h,c] / wpT[h,w] projections (weights fed pre-transposed from host)
#   hot loop over w (32 x 4-word blocks):
#     DVE: tmp[h, c] = cpT[h, c] + wpT[h, w]     (tensor_scalar, per-partition)
#     ACT: tanh(tmp) -> bf16, one 8192-wide instruction per block
#     PE:  scoresT[w, :] += Wv_chunk.T @ tanh_chunk   (4 psum-accum matvecs)
#   epilogue: exp+colsum fused on ACT, context matmuls on PE.

import numpy as np
import ml_dtypes

B, LC, LW, H = 4, 512, 256, 512
NCORES = 8
LWS = LW // 2          # words per core
PCH = H // 128         # h chunks of 128 partitions
WBLK = 4               # words per ACT instruction block

_NC_CACHE = None


def _build_bass(reps=1):
    from contextlib import ExitStack

    import concourse.mybir as mybir
    import concourse.tile as tile
    from concourse import bacc
    from concourse.masks import make_identity

    f32 = mybir.dt.float32
    bf16 = mybir.dt.bfloat16
    Alu = mybir.AluOpType
    Act = mybir.ActivationFunctionType

    nc = bacc.Bacc()

    chT_d = nc.dram_tensor("chT", [H, LC], bf16, kind="ExternalInput")
    ch_d = nc.dram_tensor("ch", [LC, H], bf16, kind="ExternalInput")
    whT_d = nc.dram_tensor("whT", [H, LWS], bf16, kind="ExternalInput")
    wh_d = nc.dram_tensor("wh", [LWS, H], bf16, kind="ExternalInput")
    WcT_d = nc.dram_tensor("WcT", [H, H], bf16, kind="ExternalInput")
    WwT_d = nc.dram_tensor("WwT", [H, H], bf16, kind="ExternalInput")
    bsum_d = nc.dram_tensor("bsum", [H], f32, kind="ExternalInput")
    Wv_d = nc.dram_tensor("Wv_bf", [H], bf16, kind="ExternalInput")
    ccp_d = nc.dram_tensor("ccp", [LC, H], f32, kind="ExternalOutput")
    rs_d = nc.dram_tensor("rs", [LC], f32, kind="ExternalOutput")
    wc_d = nc.dram_tensor("wc", [LWS, H], f32, kind="ExternalOutput")

    with tile.TileContext(nc) as tc, ExitStack() as ctx:
        persist = ctx.enter_context(tc.tile_pool(name="persist", bufs=1))
        main = ctx.enter_context(tc.tile_pool(name="main", bufs=2))
        pp = ctx.enter_context(tc.tile_pool(name="pp", bufs=1, space="PSUM"))
        pw = ctx.enter_context(tc.tile_pool(name="pw", bufs=2, space="PSUM"))

        for _rep in range(reps):
            _body(nc, tc, mybir, make_identity, persist, main, pp, pw,
                  chT_d, ch_d, whT_d, wh_d, WcT_d, WwT_d, bsum_d, Wv_d,
                  ccp_d, rs_d, wc_d)

    nc.finalize()
    return nc


def _body(nc, tc, mybir, make_identity, persist, main, pp, pw,
          chT_d, ch_d, whT_d, wh_d, WcT_d, WwT_d, bsum_d, Wv_d,
          ccp_d, rs_d, wc_d):
    f32 = mybir.dt.float32
    bf16 = mybir.dt.bfloat16
    Alu = mybir.AluOpType
    Act = mybir.ActivationFunctionType
    if True:
        # ---- input loads ----
        # Split the large loads into per-chunk DMAs so they spread across the
        # 8 HW queues; only the projection inputs gate the hot loop. ch/wh are
        # epilogue-only and issued later so they ride under the hot loop.
        bsum_sb = persist.tile([128, PCH], f32)
        nc.sync.dma_start(out=bsum_sb, in_=bsum_d.rearrange("(k p) -> p k", p=128))
        Wv_sb = persist.tile([128, PCH], bf16)
        nc.sync.dma_start(out=Wv_sb, in_=Wv_d.rearrange("(k p) -> p k", p=128))
        whT_sb = persist.tile([128, PCH, LWS], bf16)
        whT_r = whT_d.rearrange("(k p) w -> p k w", p=128)
        WwT_sb = persist.tile([128, PCH, PCH, 128], bf16)
        WwT_r = WwT_d.rearrange("(ki p) (ko q) -> p ki ko q", p=128, q=128)
        chT_sb = persist.tile([128, PCH, LC], bf16)
        chT_r = chT_d.rearrange("(k p) c -> p k c", p=128)
        WcT_sb = persist.tile([128, PCH, PCH, 128], bf16)
        WcT_r = WcT_d.rearrange("(ki p) (ko q) -> p ki ko q", p=128, q=128)
        for k in range(PCH):
            nc.sync.dma_start(out=whT_sb[:, k, :], in_=whT_r[:, k, :])
            nc.sync.dma_start(out=WwT_sb[:, k, :, :], in_=WwT_r[:, k, :, :])
            nc.sync.dma_start(out=chT_sb[:, k, :], in_=chT_r[:, k, :])
            nc.sync.dma_start(out=WcT_sb[:, k, :, :], in_=WcT_r[:, k, :, :])
        ident = persist.tile([128, 128], bf16)
        make_identity(nc, ident)

        # ---- projections: cpT[h(p), c], wpT[h(p), w] (+ summed bias) ----
        cpT_sb = persist.tile([128, PCH, LC], f32)
        for ko in range(PCH):
            ps_proj = pw.tile([128, LC], f32, tag="big")
            for ki in range(PCH):
                nc.tensor.matmul(
                    ps_proj,
                    lhsT=WcT_sb[:, ki, ko, :],
                    rhs=chT_sb[:, ki, :],
                    start=(ki == 0),
                    stop=(ki == PCH - 1),
                )
            nc.vector.tensor_copy(out=cpT_sb[:, ko, :], in_=ps_proj)
        wpT_sb = persist.tile([128, PCH, LWS], f32)
        for ko in range(PCH):
            ps_projw = pw.tile([128, LWS], f32, tag="small")
            for ki in range(PCH):
                nc.tensor.matmul(
                    ps_projw,
                    lhsT=WwT_sb[:, ki, ko, :],
                    rhs=whT_sb[:, ki, :],
                    start=(ki == 0),
                    stop=(ki == PCH - 1),
                )
            nc.vector.tensor_scalar(
                out=wpT_sb[:, ko, :],
                in0=ps_projw,
                scalar1=bsum_sb[:, ko : ko + 1],
                scalar2=None,
                op0=Alu.add,
            )

        # ---- hot loop: scores[c(p), w(f)] in psum ----
        # PE reduction over h: tanh chunk [h=128, c=128] is the stationary
        # operand (bf16 -> fast weight load), Wv chunk [128, 1] the moving one.
        scores_ps = pp.tile([128, PCH, 128], f32)  # [c(p), ct, w] -- one bank
        for wb in range(LWS // WBLK):
            tmp = main.tile([128, WBLK, PCH, LC], f32, tag="tmp")
            for wi in range(WBLK):
                w = wb * WBLK + wi
                for k in range(PCH):
                    nc.vector.tensor_scalar(
                        out=tmp[:, wi, k, :],
                        in0=cpT_sb[:, k, :],
                        scalar1=wpT_sb[:, k, w : w + 1],
                        scalar2=None,
                        op0=Alu.add,
                    )
            tanh_bf = main.tile([128, WBLK, PCH, LC], bf16, tag="tanh")
            nc.scalar.activation(out=tanh_bf, in_=tmp, func=Act.Tanh)
            for wi in range(WBLK):
                w = wb * WBLK + wi
                for ct in range(PCH):
                    for k in range(PCH):
                        nc.tensor.matmul(
                            scores_ps[:, ct, w : w + 1],
                            lhsT=tanh_bf[:, wi, k, ct * 128 : (ct + 1) * 128],
                            rhs=Wv_sb[:, k : k + 1],
                            start=(k == 0),
                            stop=(k == PCH - 1),
                        )

        # ---- epilogue ----
        # epilogue-only inputs (issued here so the prologue queues stay clear)
        ch_sb = persist.tile([128, PCH, H], bf16)
        ch_r = ch_d.rearrange("(t p) h -> p t h", p=128)
        for t in range(PCH):
            nc.sync.dma_start(out=ch_sb[:, t, :], in_=ch_r[:, t, :])
        wh_sb = persist.tile([128, H], bf16)
        nc.sync.dma_start(out=wh_sb, in_=wh_d[:, :])

        # exp over scores; char-softmax partial row sums fused via accum_out
        exp_cw_sb = persist.tile([128, PCH, 128], bf16)  # [c(p), ct, w]
        rs_sb = persist.tile([128, PCH], f32)
        for ct in range(PCH):
            nc.scalar.activation(
                out=exp_cw_sb[:, ct, :],
                in_=scores_ps[:, ct, :],
                func=Act.Exp,
                accum_out=rs_sb[:, ct : ct + 1],
            )
        nc.sync.dma_start(out=rs_d.rearrange("(t p) -> p t", p=128), in_=rs_sb)

        # transpose exp -> expT[w(p), c(f)]; word-softmax col sums fused into
        # the psum->sbuf copies via accum_out
        expT_sb = persist.tile([128, LC], bf16)
        colsum_part = persist.tile([128, PCH], f32)
        for ct in range(PCH):
            ps_tr = pw.tile([128, 128], bf16, tag="small")
            nc.tensor.transpose(ps_tr, exp_cw_sb[:, ct, :], ident)
            nc.scalar.activation(
                out=expT_sb[:, ct * 128 : (ct + 1) * 128],
                in_=ps_tr,
                func=Act.Copy,
                accum_out=colsum_part[:, ct : ct + 1],
            )
        colsum = persist.tile([128, 1], f32)
        nc.vector.reduce_sum(out=colsum, in_=colsum_part, axis=mybir.AxisListType.X)

        # char-context partial: ccp[c, :] = sum_w exp[c, w] * wh[w, :]
        ccp_sb = persist.tile([128, PCH, H], f32)
        for ct in range(PCH):
            ps_ccp = pw.tile([128, H], f32, tag="big")
            nc.tensor.matmul(
                ps_ccp,
                lhsT=expT_sb[:, ct * 128 : (ct + 1) * 128],
                rhs=wh_sb,
                start=True,
                stop=True,
            )
            nc.vector.tensor_copy(out=ccp_sb[:, ct, :], in_=ps_ccp)
        nc.sync.dma_start(
            out=ccp_d.rearrange("(t p) h -> p t h", p=128), in_=ccp_sb
        )

        # word-context (exact): wc[w, :] = sum_c (exp[c,w]/colsum[w]) * ch[c, :]
        ps_wc = pw.tile([128, H], f32, tag="big")
        for ct in range(PCH):
            nc.tensor.matmul(
                ps_wc,
                lhsT=exp_cw_sb[:, ct, :],
                rhs=ch_sb[:, ct, :],
                start=(ct == 0),
                stop=(ct == PCH - 1),
            )
        rcol = persist.tile([128, 1], f32)
        nc.vector.reciprocal(out=rcol, in_=colsum)
        wc_sb = persist.tile([128, H], f32)
        nc.vector.tensor_scalar(
            out=wc_sb, in0=ps_wc, scalar1=rcol, scalar2=None, op0=Alu.mult
        )
        nc.sync.dma_start(out=wc_d[:, :], in_=wc_sb)


def get_nc():
    global _NC_CACHE
    if _NC_CACHE is None:
        _NC_CACHE = _build_bass()
    return _NC_CACHE


def make_in_maps(char_hidden, word_hidden, Wc_w, Wc_b, Ww_b, Ww_w, Wv_w):
    """Shard + lay out the full inputs for the 8 cores (host-side, O(bytes))."""
    bf = ml_dtypes.bfloat16
    ch = np.asarray(char_hidden, dtype=np.float32).astype(bf)
    wo = np.asarray(word_hidden, dtype=np.float32).astype(bf)
    WcT = np.ascontiguousarray(np.asarray(Wc_w, dtype=np.float32).astype(bf).T)
    WwT = np.ascontiguousarray(np.asarray(Ww_w, dtype=np.float32).astype(bf).T)
    bsum = (
        np.asarray(Wc_b, dtype=np.float32) + np.asarray(Ww_b, dtype=np.float32)
    ).astype(np.float32)
    Wv_bf = np.asarray(Wv_w, dtype=np.float32).astype(bf)

    in_maps = []
    for core in range(NCORES):
        b, half = core // 2, core % 2
        chb = np.ascontiguousarray(ch[b])
        chbT = np.ascontiguousarray(ch[b].T)
        whb = np.ascontiguousarray(wo[b, half * LWS : (half + 1) * LWS])
        whbT = np.ascontiguousarray(whb.T)
        in_maps.append(
            dict(
                chT=chbT, ch=chb, whT=whbT, wh=whb,
                WcT=WcT, WwT=WwT, bsum=bsum, Wv_bf=Wv_bf,
            )
        )
    return in_maps


def combine_outputs(results):
    """Unshard: merge per-core partials into the full outputs."""
    char_context = np.empty((B, LC, H), dtype=np.float32)
    word_context = np.empty((B, LW, H), dtype=np.float32)
    for b in range(B):
        r0, r1 = results[2 * b], results[2 * b + 1]
        denom = (r0["rs"] + r1["rs"]).reshape(LC, 1)
        char_context[b] = (r0["ccp"] + r1["ccp"]) / denom
        word_context[b, :LWS] = r0["wc"]
        word_context[b, LWS:] = r1["wc"]
    return char_context, word_context


def kernel(char_hidden, word_hidden, Wc_w, Wc_b, Ww_w, Ww_b, Wv_w, Wv_b):
    # Wv_b shifts every score by the same constant; both softmaxes are
    # shift-invariant, so it cannot affect either output and is dropped.
    from concourse.bass_utils import run_bass_kernel_spmd

    in_maps = make_in_maps(char_hidden, word_hidden, Wc_w, Wc_b, Ww_b, Ww_w, Wv_w)
    res = run_bass_kernel_spmd(get_nc(), in_maps, core_ids=list(range(NCORES)))
    return combine_outputs(res.results)
